# revision 10
# baseline (speedup 1.0000x reference)
"""GATv2 Bass kernel v3 for Trainium2, 8 NeuronCores.

Problem: B=2, N=512, FIN=128, H=4, D=64 GATv2 attention (dense graph).
Sharding: one (batch, head) pair per core (B*H = 8 = n_cores).

Math per (b, h):
  h[n,d] = x W_proj (+b_proj);  u[i,e] = 0.4|a_e| (W1 h_i + bias terms)
  w[j,e] = 0.4|a_e| (W2 h_j + bias terms)
  score[i,j] = A_i + B_j + sum_e s_e |u[i,e]+w[j,e]|   (lrelu decomposition;
    A_i drops out of softmax_j; B_j = 1.5 sum_e s_e w[j,e])
  attn = softmax_j(score);  out[i,:] = attn[i,:] @ h (+bias_param)

v3 design (vs v2): the outer-sum W-mms are gone. The abs tile
  at[p=(j2,e), i] = |u[i,e] + w[j(m,j2), e]|,  j(m,j2) = jb*128 + 64*j2 + m
is computed straight from a duplicated uT tile with a per-partition
w column scalar:
  - DVE: tensor_scalar(at, uT2, wcol[:,c], 0.0, op0=add, op1=abs_max)
    (bf16 SBUF 4x perf mode), or
  - ScalarE: activation(at, uT2, Abs, bias=wcol[:,c])
split between the two engines to keep both off the critical path.
TensorE then only runs the e-reduce matmuls (sliding sred2 window, 64
accumulating MMs per 128-j score block) plus prep/epilogue:
  - exp: ScalarE Exp, per-partition bias = B_j - C (C host safety offset).
  - epilogue: acc = sum_jb eeT_jb^T @ [h_jb|1] (hi+lo bf16 passes); the
    ones column gives Z_i, so softmax needs no row-max/zsum DVE work.
"""

import numpy as np
import ml_dtypes

import concourse.bacc as bacc
import concourse.mybir as mybir
import concourse.tile as tile
from concourse.bass_utils import run_bass_kernel_spmd

F32 = mybir.dt.float32
BF16 = mybir.dt.bfloat16
BF = ml_dtypes.bfloat16

B, N, FIN, H, D = 2, 512, 128, 4, 64
NEG_SLOPE = 0.2
C_ABS = (1.0 - NEG_SLOPE) / 2.0   # 0.4
E = D                             # 64
NB = N // 128                     # 4 j-blocks / i-blocks
PPJ = 64                          # pair-chunks per j-block
NPAIR = NB * PPJ                  # 256 chunks total
SRW = 191                         # sred2 sliding-window region width
LAG = 10                          # reduce-mms trail abs ops by LAG chunks
EXPLAG = 6                        # exp(jb) trails block jb's last MM


def _abs_engine(g):
    # 3-way split tuned from trace: DVE ~407ns/pair effective, ScalarE
    # ~790ns, GpSimd ~1us; TensorE reduce floor ~57us sets the target
    r = g % 8
    if r in (5, 7):
        return "scalar"
    if r == 2:
        return "gpsimd"
    return "vector"

last_results = None
_cache = {}


def _build(use_bias_param):
    nc = bacc.Bacc("TRN2", target_bir_lowering=False, debug=False, num_devices=8)

    x_d = nc.dram_tensor("x", [N, FIN], F32, kind="ExternalInput")
    mp_d = nc.dram_tensor("mpack", [FIN, 2 * E + D], BF16, kind="ExternalInput")
    id128_d = nc.dram_tensor("id128", [128, 128], F32, kind="ExternalInput")
    sred_d = nc.dram_tensor("sred", [128, SRW], BF16, kind="ExternalInput")
    sp_d = nc.dram_tensor("spack", [128, 4], F32, kind="ExternalInput")
    if use_bias_param:
        bprm_d = nc.dram_tensor("biasprm", [128, D], F32, kind="ExternalInput")
    out_d = nc.dram_tensor("out", [N, D], F32, kind="ExternalOutput")

    AF = mybir.ActivationFunctionType
    ALU = mybir.AluOpType

    with tile.TileContext(nc) as tc:
        with tc.tile_pool(name="sb", bufs=1) as sb:
            # ---------- persistent SBUF tiles ----------
            xb = sb.tile([128, NB * 128], F32)
            xT = sb.tile([128, N], F32)
            xTb = sb.tile([128, N], BF16)
            mpack = sb.tile([FIN, 2 * E + D], BF16)
            id128 = sb.tile([128, 128], F32)
            uT2 = sb.tile([128, N], BF16)     # rows 0:64 uT; rows 64:128 dup
            wTf = sb.tile([E, N], F32)
            wcol = sb.tile([128, NPAIR], F32)  # col jb*64+m = w pair column
            sred = sb.tile([128, SRW], BF16)
            spack = sb.tile([128, 4], F32)
            bexp = sb.tile([128, NB], F32)
            hx_hi = sb.tile([128, NB * (D + 1)], BF16)
            hx_lo = sb.tile([128, NB * (D + 1)], BF16)
            eeT = [sb.tile([128, N], BF16, tag=f"ee{jb}", name=f"ee{jb}")
                   for jb in range(NB)]
            rzt = sb.tile([128, NB], F32)
            o = [sb.tile([128, D], F32, tag=f"o{ib}", name=f"o{ib}")
                 for ib in range(NB)]
            if use_bias_param:
                bprm = sb.tile([128, D], F32)

            # ---------- input DMAs ----------
            nc.sync.dma_start(id128[:], id128_d.ap())
            nc.sync.dma_start(mpack[:], mp_d.ap())
            nc.sync.dma_start(spack[:], sp_d.ap())
            for nb in range(NB):
                nc.sync.dma_start(
                    xb[:, nb * 128:(nb + 1) * 128],
                    x_d.ap()[nb * 128:(nb + 1) * 128, :])
            nc.sync.dma_start(sred[:], sred_d.ap())
            if use_bias_param:
                nc.sync.dma_start(bprm[:], bprm_d.ap())

            # hx ones/zero columns
            nc.scalar.memzero(hx_hi[:, :])
            nc.scalar.memzero(hx_lo[:, :])
            for jb in range(NB):
                cc = jb * (D + 1) + D
                nc.scalar.add(hx_hi[:, cc:cc + 1], hx_hi[:, cc:cc + 1], 1.0)

            # ---------- prep ----------
            m1 = mpack[:, 0:E]
            m2 = mpack[:, E:2 * E]
            wp = mpack[:, 2 * E:2 * E + D]
            with tc.tile_pool(name="pp", bufs=4, space="PSUM") as pp:
                for nb in range(NB):
                    t = pp.tile([128, 512], F32, tag="t", name="t")
                    nc.tensor.transpose(t[:, 0:128], xb[:, nb * 128:(nb + 1) * 128],
                                        id128[:])
                    nc.scalar.copy(xT[:, nb * 128:(nb + 1) * 128], t[:, 0:128])
                    nc.vector.tensor_reduce(
                        xTb[:, nb * 128:(nb + 1) * 128],
                        xT[:, nb * 128:(nb + 1) * 128].rearrange(
                            "p (f o) -> p f o", o=1),
                        axis=mybir.AxisListType.X, op=ALU.max)
                for nb in range(NB):
                    s_ = slice(nb * 128, (nb + 1) * 128)
                    # uT block (bias = ub)
                    t = pp.tile([128, 512], F32, tag="t", name="t")
                    nc.tensor.matmul(t[0:E, 0:128], m1, xTb[:, s_],
                                     start=True, stop=True)
                    nc.scalar.activation(uT2[0:E, s_], t[0:E, 0:128],
                                         AF.Identity, bias=spack[0:E, 1:2])
                # duplicate uT into rows 64:128 (SBUF->SBUF DMA)
                nc.sync.dma_start(uT2[E:2 * E, :], uT2[0:E, :])
                for nb in range(NB):
                    s_ = slice(nb * 128, (nb + 1) * 128)
                    # wT block (f32; bias = wb)
                    t2 = pp.tile([128, 512], F32, tag="t", name="t2")
                    nc.tensor.matmul(t2[0:E, 0:128], m2, xTb[:, s_],
                                     start=True, stop=True)
                    nc.scalar.activation(wTf[:, s_], t2[0:E, 0:128],
                                         AF.Identity, bias=spack[0:E, 2:3])
                    # wcol[(j2,e), nb*64+m] = wTf[e, nb*128 + 64*j2 + m]
                    for j2 in range(2):
                        nc.sync.dma_start(
                            wcol[j2 * E:(j2 + 1) * E,
                                 nb * PPJ:(nb + 1) * PPJ],
                            wTf[:, nb * 128 + j2 * PPJ:
                                nb * 128 + (j2 + 1) * PPJ])
                # B_j columns -> bexp[:, jb] = B - C
                for jb in range(NB):
                    t = pp.tile([128, 512], F32, tag="t", name="t")
                    nc.tensor.matmul(t[:, 0:1], wTf[:, jb * 128:(jb + 1) * 128],
                                     spack[0:E, 0:1], start=True, stop=True)
                    nc.vector.scalar_tensor_tensor(
                        bexp[:, jb:jb + 1], t[:, 0:1], 1.0, spack[:, 3:4],
                        op0=ALU.mult, op1=ALU.add)
                # h row blocks -> hx_hi / hx_lo (bf16 split for accuracy)
                for nb in range(NB):
                    t = pp.tile([128, 512], F32, tag="t", name="t")
                    nc.tensor.matmul(t[:, 0:D], xTb[:, nb * 128:(nb + 1) * 128],
                                     wp, start=True, stop=True)
                    c0 = nb * (D + 1)
                    nc.scalar.copy(hx_hi[:, c0:c0 + D], t[:, 0:D])
                    nc.vector.tensor_tensor(
                        hx_lo[:, c0:c0 + D], t[:, 0:D], hx_hi[:, c0:c0 + D],
                        op=ALU.subtract)

            # ---------- main loop ----------
            # abs tile (DVE/ScalarE) -> e-reduce matmul trailing by LAG;
            # exp(jb) trails block jb's stop-MM by EXPLAG chunks so the
            # ScalarE FIFO never stalls on the PSUM dependency.
            with tc.tile_pool(name="scp", bufs=2, space="PSUM") as scpool, \
                 tc.tile_pool(name="ap", bufs=LAG + 8) as apool:
                scb = {}
                atiles = {}
                exp_at = {}

                def emit_abs(g):
                    at = apool.tile([128, 512], BF16, tag="a", name=f"a{g}")
                    atiles[g] = at
                    # relu(u+w); |z| folded via |z| = 2 relu(z) - z with the
                    # linear part rank-1 (A'_i drops, B'_j in bexp coef)
                    e_ = _abs_engine(g)
                    if e_ == "scalar":
                        nc.scalar.activation(at[:], uT2[:], AF.Relu,
                                             bias=wcol[:, g:g + 1])
                    else:
                        eng = nc.vector if e_ == "vector" else nc.gpsimd
                        with nc.allow_low_precision(reason="bf16 relu tile"):
                            eng.tensor_scalar(
                                at[:], uT2[:], wcol[:, g:g + 1], 0.0,
                                op0=ALU.add, op1=ALU.max)

                def emit_red(g):
                    jb, m = divmod(g, PPJ)
                    if m == 0:
                        scb[jb] = scpool.tile([128, 512], F32, tag="sc",
                                              name=f"sc{jb}")
                    at = atiles.pop(g)
                    nc.tensor.matmul(
                        scb[jb][:],
                        sred[:, PPJ - 1 - m:SRW - m],
                        at[:],
                        start=(m == 0), stop=(m == PPJ - 1),
                        skip_group_check=True)
                    if m == PPJ - 1:
                        exp_at[g + LAG + EXPLAG] = jb

                def emit_exp(jb):
                    nc.scalar.activation(eeT[jb][:], scb[jb][:], AF.Exp,
                                         bias=bexp[:, jb:jb + 1])

                for g in range(NPAIR + LAG + EXPLAG + 1):
                    if g < NPAIR:
                        emit_abs(g)
                    if LAG <= g < NPAIR + LAG:
                        emit_red(g - LAG)
                    if g in exp_at:
                        emit_exp(exp_at.pop(g))

            # ---------- epilogue: out = (eeT^T @ [h|1]) * 1/Z ----------
            with tc.tile_pool(name="ep", bufs=4, space="PSUM") as ep:
                for ib in range(NB):
                    acc = ep.tile([128, D + 1], F32, tag="acc", name=f"acc{ib}")
                    for jb in range(NB):
                        c0 = jb * (D + 1)
                        nc.tensor.matmul(
                            acc[:], eeT[jb][:, ib * 128:(ib + 1) * 128],
                            hx_hi[:, c0:c0 + D + 1],
                            start=(jb == 0), stop=False)
                        nc.tensor.matmul(
                            acc[:], eeT[jb][:, ib * 128:(ib + 1) * 128],
                            hx_lo[:, c0:c0 + D + 1],
                            start=False, stop=(jb == NB - 1))
                    nc.vector.reciprocal(rzt[:, ib:ib + 1], acc[:, D:D + 1])
                    nc.scalar.activation(o[ib][:], acc[:, 0:D], AF.Copy,
                                         bias=0.0, scale=rzt[:, ib:ib + 1])
                    if use_bias_param:
                        nc.gpsimd.tensor_tensor(o[ib][:], o[ib][:], bprm[:],
                                                op=ALU.add)
                    nc.sync.dma_start(out_d.ap()[ib * 128:(ib + 1) * 128, :],
                                      o[ib][:])

    nc.compile()
    return nc


def kernel(x, W_proj, b_proj, W_cat_weight, W_cat_bias, a, bias_param):
    global last_results
    x = np.asarray(x, dtype=np.float32)
    W_proj = np.asarray(W_proj, dtype=np.float32)
    b_proj = np.asarray(b_proj, dtype=np.float32)
    W_cat_weight = np.asarray(W_cat_weight, dtype=np.float32)
    W_cat_bias = np.asarray(W_cat_bias, dtype=np.float32)
    a = np.asarray(a, dtype=np.float32)
    bias_param = np.asarray(bias_param, dtype=np.float32)

    W1 = W_cat_weight[:, :, :D]
    W2 = W_cat_weight[:, :, D:]

    use_bias_param = bool(np.any(bias_param))
    key = (use_bias_param,)
    if key not in _cache:
        _cache[key] = _build(*key)
    nc = _cache[key]

    id128 = np.eye(128, dtype=np.float32)

    in_maps = []
    for c in range(8):
        b, h = divmod(c, H)
        ah = a[h]
        scale = C_ABS * np.abs(ah)
        sgn = np.sign(ah).astype(np.float32)
        M1 = (W1[h] * scale[:, None]) @ W_proj[h].T       # [E, FIN]
        M2 = (W2[h] * scale[:, None]) @ W_proj[h].T
        ub = (W1[h] * scale[:, None]) @ b_proj[h]         # [E]
        wb = scale * W_cat_bias[h] + (W2[h] * scale[:, None]) @ b_proj[h]

        # sred2: window for chunk m is sred[:, 63-m : 191-m]; partition
        # p=(j2,e) hits output row m + 64*j2 with weight sgn[e]
        sredw = np.zeros((128, SRW), dtype=np.float32)
        p = np.arange(128)
        sredw[p, PPJ - 1 + PPJ * (p // E)] = 2.0 * sgn[p % E]

        # safety offset C for exp (A_i is never added; bound the rest)
        u_full = x[b] @ M1.T + ub
        w_full = x[b] @ M2.T + wb
        B_full = 1.5 * (w_full @ sgn)
        bound = (B_full.max() + np.abs(u_full).max(axis=0).sum()
                 + np.abs(w_full).max(axis=0).sum())
        C = float(max(0.0, bound - 70.0))

        # mpack: [M1.T (64) | M2.T (64) | W_proj (64)]  as [FIN, .]
        mpk = np.zeros((FIN, 2 * E + D), dtype=np.float32)
        mpk[:, 0:E] = M1.T
        mpk[:, E:2 * E] = M2.T
        mpk[:, 2 * E:] = W_proj[h]
        # spack: col0 = 0.5*sgn (B_j - B'_j coef), col1 = ub, col2 = wb,
        # col3 = -C
        spk = np.zeros((128, 4), dtype=np.float32)
        spk[:E, 0] = 0.5 * sgn
        spk[:E, 1] = ub
        spk[:E, 2] = wb
        spk[:, 3] = -C

        mmap = {
            "x": np.ascontiguousarray(x[b]),
            "mpack": mpk.astype(BF),
            "id128": id128,
            "sred": sredw.astype(BF),
            "spack": spk,
        }
        if use_bias_param:
            mmap["biasprm"] = np.tile(bias_param[None, h * D:(h + 1) * D],
                                      (128, 1)).astype(np.float32)
        in_maps.append(mmap)

    res = run_bass_kernel_spmd(nc, in_maps, core_ids=list(range(8)))
    last_results = res

    out = np.empty((B, N, H * D), dtype=np.float32)
    for c in range(8):
        b, h = divmod(c, H)
        out[b, :, h * D:(h + 1) * D] = res.results[c]["out"]
    return out


# revision 11
# speedup vs baseline: 3.4862x; 3.4862x over previous
"""GATv2 Bass kernel v3 for Trainium2, 8 NeuronCores.

Problem: B=2, N=512, FIN=128, H=4, D=64 GATv2 attention (dense graph).
Sharding: one (batch, head) pair per core (B*H = 8 = n_cores).

Math per (b, h):
  h[n,d] = x W_proj (+b_proj);  u[i,e] = 0.4|a_e| (W1 h_i + bias terms)
  w[j,e] = 0.4|a_e| (W2 h_j + bias terms)
  score[i,j] = A_i + B_j + sum_e s_e |u[i,e]+w[j,e]|   (lrelu decomposition;
    A_i drops out of softmax_j; B_j = 1.5 sum_e s_e w[j,e])
  attn = softmax_j(score);  out[i,:] = attn[i,:] @ h (+bias_param)

v3 design (vs v2): the outer-sum W-mms are gone. The abs tile
  at[p=(j2,e), i] = |u[i,e] + w[j(m,j2), e]|,  j(m,j2) = jb*128 + 64*j2 + m
is computed straight from a duplicated uT tile with a per-partition
w column scalar:
  - DVE: tensor_scalar(at, uT2, wcol[:,c], 0.0, op0=add, op1=abs_max)
    (bf16 SBUF 4x perf mode), or
  - ScalarE: activation(at, uT2, Abs, bias=wcol[:,c])
split between the two engines to keep both off the critical path.
TensorE then only runs the e-reduce matmuls (sliding sred2 window, 64
accumulating MMs per 128-j score block) plus prep/epilogue:
  - exp: ScalarE Exp, per-partition bias = B_j - C (C host safety offset).
  - epilogue: acc = sum_jb eeT_jb^T @ [h_jb|1] (hi+lo bf16 passes); the
    ones column gives Z_i, so softmax needs no row-max/zsum DVE work.
"""

import numpy as np
import ml_dtypes

import concourse.bacc as bacc
import concourse.mybir as mybir
import concourse.tile as tile
from concourse.bass_utils import run_bass_kernel_spmd

F32 = mybir.dt.float32
BF16 = mybir.dt.bfloat16
BF = ml_dtypes.bfloat16

B, N, FIN, H, D = 2, 512, 128, 4, 64
NEG_SLOPE = 0.2
C_ABS = (1.0 - NEG_SLOPE) / 2.0   # 0.4
E = D                             # 64
NB = N // 128                     # 4 j-blocks / i-blocks
PPJ = 64                          # pair-chunks per j-block
NPAIR = NB * PPJ                  # 256 chunks total
SRW = 191                         # sred2 sliding-window region width
LAG = 10                          # reduce-mms trail abs ops by LAG chunks
EXPLAG = 6                        # exp(jb) trails block jb's last MM


def _abs_engine(g):
    # 3-way split tuned from trace: DVE ~407ns/pair effective, ScalarE
    # ~790ns, GpSimd ~1us; TensorE reduce floor ~57us sets the target
    # gpsimd tensor_scalar measured ~10x too slow on HW -- 2-way only
    if g % 4 == 3:
        return "scalar"
    return "vector"

last_results = None
_cache = {}


def _build(use_bias_param):
    nc = bacc.Bacc("TRN2", target_bir_lowering=False, debug=False, num_devices=8)

    x_d = nc.dram_tensor("x", [N, FIN], F32, kind="ExternalInput")
    mp_d = nc.dram_tensor("mpack", [FIN, 2 * E + D], BF16, kind="ExternalInput")
    id128_d = nc.dram_tensor("id128", [128, 128], F32, kind="ExternalInput")
    sred_d = nc.dram_tensor("sred", [128, SRW], BF16, kind="ExternalInput")
    sp_d = nc.dram_tensor("spack", [128, 4], F32, kind="ExternalInput")
    if use_bias_param:
        bprm_d = nc.dram_tensor("biasprm", [128, D], F32, kind="ExternalInput")
    out_d = nc.dram_tensor("out", [N, D], F32, kind="ExternalOutput")

    AF = mybir.ActivationFunctionType
    ALU = mybir.AluOpType

    with tile.TileContext(nc) as tc:
        with tc.tile_pool(name="sb", bufs=1) as sb:
            # ---------- persistent SBUF tiles ----------
            xb = sb.tile([128, NB * 128], F32)
            xT = sb.tile([128, N], F32)
            xTb = sb.tile([128, N], BF16)
            mpack = sb.tile([FIN, 2 * E + D], BF16)
            id128 = sb.tile([128, 128], F32)
            uT2 = sb.tile([128, N], BF16)     # rows 0:64 uT; rows 64:128 dup
            wTf = sb.tile([E, N], F32)
            wcol = sb.tile([128, NPAIR], F32)  # col jb*64+m = w pair column
            sred = sb.tile([128, SRW], BF16)
            spack = sb.tile([128, 4], F32)
            bexp = sb.tile([128, NB], F32)
            hx_hi = sb.tile([128, NB * (D + 1)], BF16)
            hx_lo = sb.tile([128, NB * (D + 1)], BF16)
            eeT = [sb.tile([128, N], BF16, tag=f"ee{jb}", name=f"ee{jb}")
                   for jb in range(NB)]
            rzt = sb.tile([128, NB], F32)
            o = [sb.tile([128, D], F32, tag=f"o{ib}", name=f"o{ib}")
                 for ib in range(NB)]
            if use_bias_param:
                bprm = sb.tile([128, D], F32)

            # ---------- input DMAs ----------
            nc.sync.dma_start(id128[:], id128_d.ap())
            nc.sync.dma_start(mpack[:], mp_d.ap())
            nc.sync.dma_start(spack[:], sp_d.ap())
            for nb in range(NB):
                nc.sync.dma_start(
                    xb[:, nb * 128:(nb + 1) * 128],
                    x_d.ap()[nb * 128:(nb + 1) * 128, :])
            nc.sync.dma_start(sred[:], sred_d.ap())
            if use_bias_param:
                nc.sync.dma_start(bprm[:], bprm_d.ap())

            # hx ones/zero columns
            nc.scalar.memzero(hx_hi[:, :])
            nc.scalar.memzero(hx_lo[:, :])
            for jb in range(NB):
                cc = jb * (D + 1) + D
                nc.scalar.add(hx_hi[:, cc:cc + 1], hx_hi[:, cc:cc + 1], 1.0)

            # ---------- prep ----------
            m1 = mpack[:, 0:E]
            m2 = mpack[:, E:2 * E]
            wp = mpack[:, 2 * E:2 * E + D]
            with tc.tile_pool(name="pp", bufs=4, space="PSUM") as pp:
                for nb in range(NB):
                    t = pp.tile([128, 512], F32, tag="t", name="t")
                    nc.tensor.transpose(t[:, 0:128], xb[:, nb * 128:(nb + 1) * 128],
                                        id128[:])
                    nc.scalar.copy(xT[:, nb * 128:(nb + 1) * 128], t[:, 0:128])
                    nc.vector.tensor_reduce(
                        xTb[:, nb * 128:(nb + 1) * 128],
                        xT[:, nb * 128:(nb + 1) * 128].rearrange(
                            "p (f o) -> p f o", o=1),
                        axis=mybir.AxisListType.X, op=ALU.max)
                for nb in range(NB):
                    s_ = slice(nb * 128, (nb + 1) * 128)
                    # uT block (bias = ub)
                    t = pp.tile([128, 512], F32, tag="t", name="t")
                    nc.tensor.matmul(t[0:E, 0:128], m1, xTb[:, s_],
                                     start=True, stop=True)
                    nc.scalar.activation(uT2[0:E, s_], t[0:E, 0:128],
                                         AF.Identity, bias=spack[0:E, 1:2])
                # duplicate uT into rows 64:128 (SBUF->SBUF DMA)
                nc.sync.dma_start(uT2[E:2 * E, :], uT2[0:E, :])
                for nb in range(NB):
                    s_ = slice(nb * 128, (nb + 1) * 128)
                    # wT block (f32; bias = wb)
                    t2 = pp.tile([128, 512], F32, tag="t", name="t2")
                    nc.tensor.matmul(t2[0:E, 0:128], m2, xTb[:, s_],
                                     start=True, stop=True)
                    nc.scalar.activation(wTf[:, s_], t2[0:E, 0:128],
                                         AF.Identity, bias=spack[0:E, 2:3])
                    # wcol[(j2,e), nb*64+m] = wTf[e, nb*128 + 64*j2 + m]
                    for j2 in range(2):
                        nc.sync.dma_start(
                            wcol[j2 * E:(j2 + 1) * E,
                                 nb * PPJ:(nb + 1) * PPJ],
                            wTf[:, nb * 128 + j2 * PPJ:
                                nb * 128 + (j2 + 1) * PPJ])
                # B_j columns -> bexp[:, jb] = B - C
                for jb in range(NB):
                    t = pp.tile([128, 512], F32, tag="t", name="t")
                    nc.tensor.matmul(t[:, 0:1], wTf[:, jb * 128:(jb + 1) * 128],
                                     spack[0:E, 0:1], start=True, stop=True)
                    nc.vector.scalar_tensor_tensor(
                        bexp[:, jb:jb + 1], t[:, 0:1], 1.0, spack[:, 3:4],
                        op0=ALU.mult, op1=ALU.add)
                # h row blocks -> hx_hi / hx_lo (bf16 split for accuracy)
                for nb in range(NB):
                    t = pp.tile([128, 512], F32, tag="t", name="t")
                    nc.tensor.matmul(t[:, 0:D], xTb[:, nb * 128:(nb + 1) * 128],
                                     wp, start=True, stop=True)
                    c0 = nb * (D + 1)
                    nc.scalar.copy(hx_hi[:, c0:c0 + D], t[:, 0:D])
                    nc.vector.tensor_tensor(
                        hx_lo[:, c0:c0 + D], t[:, 0:D], hx_hi[:, c0:c0 + D],
                        op=ALU.subtract)

            # ---------- main loop ----------
            # abs tile (DVE/ScalarE) -> e-reduce matmul trailing by LAG;
            # exp(jb) trails block jb's stop-MM by EXPLAG chunks so the
            # ScalarE FIFO never stalls on the PSUM dependency.
            with tc.tile_pool(name="scp", bufs=2, space="PSUM") as scpool, \
                 tc.tile_pool(name="ap", bufs=LAG + 8) as apool:
                scb = {}
                atiles = {}
                exp_at = {}

                def emit_abs(g):
                    at = apool.tile([128, 512], BF16, tag="a", name=f"a{g}")
                    atiles[g] = at
                    # relu(u+w); |z| folded via |z| = 2 relu(z) - z with the
                    # linear part rank-1 (A'_i drops, B'_j in bexp coef)
                    e_ = _abs_engine(g)
                    if e_ == "scalar":
                        nc.scalar.activation(at[:], uT2[:], AF.Relu,
                                             bias=wcol[:, g:g + 1])
                    else:
                        eng = nc.vector if e_ == "vector" else nc.gpsimd
                        with nc.allow_low_precision(reason="bf16 relu tile"):
                            eng.tensor_scalar(
                                at[:], uT2[:], wcol[:, g:g + 1], 0.0,
                                op0=ALU.add, op1=ALU.max)

                def emit_red(g):
                    jb, m = divmod(g, PPJ)
                    if m == 0:
                        scb[jb] = scpool.tile([128, 512], F32, tag="sc",
                                              name=f"sc{jb}")
                    at = atiles.pop(g)
                    nc.tensor.matmul(
                        scb[jb][:],
                        sred[:, PPJ - 1 - m:SRW - m],
                        at[:],
                        start=(m == 0), stop=(m == PPJ - 1),
                        skip_group_check=True)
                    if m == PPJ - 1:
                        exp_at[g + LAG + EXPLAG] = jb

                def emit_exp(jb):
                    nc.scalar.activation(eeT[jb][:], scb[jb][:], AF.Exp,
                                         bias=bexp[:, jb:jb + 1])

                for g in range(NPAIR + LAG + EXPLAG + 1):
                    if g < NPAIR:
                        emit_abs(g)
                    if LAG <= g < NPAIR + LAG:
                        emit_red(g - LAG)
                    if g in exp_at:
                        emit_exp(exp_at.pop(g))

            # ---------- epilogue: out = (eeT^T @ [h|1]) * 1/Z ----------
            with tc.tile_pool(name="ep", bufs=4, space="PSUM") as ep:
                for ib in range(NB):
                    acc = ep.tile([128, D + 1], F32, tag="acc", name=f"acc{ib}")
                    for jb in range(NB):
                        c0 = jb * (D + 1)
                        nc.tensor.matmul(
                            acc[:], eeT[jb][:, ib * 128:(ib + 1) * 128],
                            hx_hi[:, c0:c0 + D + 1],
                            start=(jb == 0), stop=False)
                        nc.tensor.matmul(
                            acc[:], eeT[jb][:, ib * 128:(ib + 1) * 128],
                            hx_lo[:, c0:c0 + D + 1],
                            start=False, stop=(jb == NB - 1))
                    nc.vector.reciprocal(rzt[:, ib:ib + 1], acc[:, D:D + 1])
                    nc.scalar.activation(o[ib][:], acc[:, 0:D], AF.Copy,
                                         bias=0.0, scale=rzt[:, ib:ib + 1])
                    if use_bias_param:
                        nc.gpsimd.tensor_tensor(o[ib][:], o[ib][:], bprm[:],
                                                op=ALU.add)
                    nc.sync.dma_start(out_d.ap()[ib * 128:(ib + 1) * 128, :],
                                      o[ib][:])

    nc.compile()
    return nc


def kernel(x, W_proj, b_proj, W_cat_weight, W_cat_bias, a, bias_param):
    global last_results
    x = np.asarray(x, dtype=np.float32)
    W_proj = np.asarray(W_proj, dtype=np.float32)
    b_proj = np.asarray(b_proj, dtype=np.float32)
    W_cat_weight = np.asarray(W_cat_weight, dtype=np.float32)
    W_cat_bias = np.asarray(W_cat_bias, dtype=np.float32)
    a = np.asarray(a, dtype=np.float32)
    bias_param = np.asarray(bias_param, dtype=np.float32)

    W1 = W_cat_weight[:, :, :D]
    W2 = W_cat_weight[:, :, D:]

    use_bias_param = bool(np.any(bias_param))
    key = (use_bias_param,)
    if key not in _cache:
        _cache[key] = _build(*key)
    nc = _cache[key]

    id128 = np.eye(128, dtype=np.float32)

    in_maps = []
    for c in range(8):
        b, h = divmod(c, H)
        ah = a[h]
        scale = C_ABS * np.abs(ah)
        sgn = np.sign(ah).astype(np.float32)
        M1 = (W1[h] * scale[:, None]) @ W_proj[h].T       # [E, FIN]
        M2 = (W2[h] * scale[:, None]) @ W_proj[h].T
        ub = (W1[h] * scale[:, None]) @ b_proj[h]         # [E]
        wb = scale * W_cat_bias[h] + (W2[h] * scale[:, None]) @ b_proj[h]

        # sred2: window for chunk m is sred[:, 63-m : 191-m]; partition
        # p=(j2,e) hits output row m + 64*j2 with weight sgn[e]
        sredw = np.zeros((128, SRW), dtype=np.float32)
        p = np.arange(128)
        sredw[p, PPJ - 1 + PPJ * (p // E)] = 2.0 * sgn[p % E]

        # safety offset C for exp (A_i is never added; bound the rest)
        u_full = x[b] @ M1.T + ub
        w_full = x[b] @ M2.T + wb
        B_full = 1.5 * (w_full @ sgn)
        bound = (B_full.max() + np.abs(u_full).max(axis=0).sum()
                 + np.abs(w_full).max(axis=0).sum())
        C = float(max(0.0, bound - 70.0))

        # mpack: [M1.T (64) | M2.T (64) | W_proj (64)]  as [FIN, .]
        mpk = np.zeros((FIN, 2 * E + D), dtype=np.float32)
        mpk[:, 0:E] = M1.T
        mpk[:, E:2 * E] = M2.T
        mpk[:, 2 * E:] = W_proj[h]
        # spack: col0 = 0.5*sgn (B_j - B'_j coef), col1 = ub, col2 = wb,
        # col3 = -C
        spk = np.zeros((128, 4), dtype=np.float32)
        spk[:E, 0] = 0.5 * sgn
        spk[:E, 1] = ub
        spk[:E, 2] = wb
        spk[:, 3] = -C

        mmap = {
            "x": np.ascontiguousarray(x[b]),
            "mpack": mpk.astype(BF),
            "id128": id128,
            "sred": sredw.astype(BF),
            "spack": spk,
        }
        if use_bias_param:
            mmap["biasprm"] = np.tile(bias_param[None, h * D:(h + 1) * D],
                                      (128, 1)).astype(np.float32)
        in_maps.append(mmap)

    res = run_bass_kernel_spmd(nc, in_maps, core_ids=list(range(8)))
    last_results = res

    out = np.empty((B, N, H * D), dtype=np.float32)
    for c in range(8):
        b, h = divmod(c, H)
        out[b, :, h * D:(h + 1) * D] = res.results[c]["out"]
    return out
